# revision 1
# baseline (speedup 1.0000x reference)
"""Trainium2 Bass kernel for nn_DisRNNCellNet (time-decayed LSTM + noisy-OR).

Data-parallel over 8 NeuronCores: bsize 4096 -> 512/core (4096 flat samples
per core, incl. the 8 nodules). Per core a 32-step LSTM (hid=64) runs with
features on SBUF partitions and samples on the free dim, batch split in two
halves of 2048 that share 128-partition-dense ACT/DVE ops:

  pif_h0 [128,2048] = (f,i) gate preacts of half0; pif_h1 = (i,f) of half1
  tg2    [128,2048] = g preacts: rows 0:64 half1, 64:128 half0 (M=64 MMs)
  poo    [128,2048] = o preacts: rows 0:64 half0, 64:128 half1
  c2     [128,2048] = cell state: rows 0:64 half0, 64:128 half1

  ACT (all dense):  sig(pif0) sig(pif1) tanh(tg2) sig(poo) tanh(c2)
  DVE: dc2=c2*dec2 | ig,fdc per half (bases matched) | add | h per half

The gate permutations exist so every 2-input DVE op sees equal input base
partitions (walrus checkSBSameStartPartition). Decay 1/log(e+dt) is host-
precomputed, host-replicated over 64 partitions. Final FC + noisy-OR pooling
on-device.
"""

import math

import ml_dtypes
import numpy as np

import concourse.bass as bass
import concourse.mybir as mybir
import concourse.tile as tile
from concourse.bass_utils import run_bass_kernel_spmd

BF16 = mybir.dt.bfloat16
F32 = mybir.dt.float32
AF = mybir.ActivationFunctionType

STEP, BSIZE, NNOD, DIM, HID = 32, 4096, 8, 64, 64
NCORES = 8
BL = (BSIZE // NCORES) * NNOD  # 4096 flat samples per core
HALF = BL // 2  # 2048
NB = HALF // 512  # 512-wide matmul chunks per half

LAST_RESULT = None


def _split_multiwaits(nc, max_waits=1):
    """walrus in this env rejects >1 sem wait per instruction ("Too many
    sync wait commands"); split extras onto single-wait NoOps."""
    for bb in nc.main_func.blocks:
        out = []
        for ins in bb.instructions:
            si = ins.sync_info
            if si is not None and len(si.on_wait) > max_waits:
                waits = list(si.on_wait)
                for j, w in enumerate(waits[:-max_waits]):
                    out.append(
                        mybir.InstNoOp(
                            name=f"{ins.name}-wsplit{j}",
                            engine=ins.engine,
                            ins=[],
                            outs=[],
                            sync_info=mybir.SyncInfo(on_wait=[w], on_update=[]),
                        )
                    )
                ins.sync_info = mybir.SyncInfo(
                    on_wait=waits[-max_waits:], on_update=list(si.on_update)
                )
            out.append(ins)
        bb.instructions = out


def _build(fc2_b: float, k_base: float):
    nc = bass.Bass(target_bir_lowering=False)
    x_d = nc.declare_dram_parameter("x", [STEP, DIM, BL], BF16, isOutput=False)
    dec_d = nc.declare_dram_parameter("dec", [STEP, 128, HALF], BF16, isOutput=False)
    wfi_d = nc.declare_dram_parameter("wfi", [128, 128], BF16, isOutput=False)
    wif_d = nc.declare_dram_parameter("wif", [128, 128], BF16, isOutput=False)
    wg_d = nc.declare_dram_parameter("wg", [128, HID], BF16, isOutput=False)
    wo_d = nc.declare_dram_parameter("wo", [128, HID], BF16, isOutput=False)
    bfi_d = nc.declare_dram_parameter("bfi", [128, 1], F32, isOutput=False)
    bif_d = nc.declare_dram_parameter("bif", [128, 1], F32, isOutput=False)
    bg_d = nc.declare_dram_parameter("bg", [128, 1], F32, isOutput=False)
    bo_d = nc.declare_dram_parameter("bo", [128, 1], F32, isOutput=False)
    fc2_d = nc.declare_dram_parameter("fc2w", [HID, 1], BF16, isOutput=False)
    out_d = nc.declare_dram_parameter("out", [1, BSIZE // NCORES], F32, isOutput=True)

    with tile.TileContext(nc) as tc:
        with (
            tc.tile_pool(name="const", bufs=1) as const,
            tc.tile_pool(name="decp", bufs=2) as decp,
            tc.tile_pool(name="work", bufs=3) as work,
            tc.tile_pool(name="psum", bufs=1, space="PSUM") as psum,
        ):
            wfi = const.tile([128, 128], BF16, tag="wfi", name="wfi")
            wif = const.tile([128, 128], BF16, tag="wif", name="wif")
            wg = const.tile([128, HID], BF16, tag="wg", name="wg")
            wo = const.tile([128, HID], BF16, tag="wo", name="wo")
            bfi = const.tile([128, 1], F32, tag="bfi", name="bfi")
            bif = const.tile([128, 1], F32, tag="bif", name="bif")
            bg = const.tile([128, 1], F32, tag="bg", name="bg")
            bo = const.tile([128, 1], F32, tag="bo", name="bo")
            fc2 = const.tile([HID, 1], BF16, tag="fc2", name="fc2")
            for sb, dr in [
                (wfi, wfi_d), (wif, wif_d), (wg, wg_d), (wo, wo_d),
                (bfi, bfi_d), (bif, bif_d), (bg, bg_d), (bo, bo_d), (fc2, fc2_d),
            ]:
                nc.sync.dma_start(out=sb[:], in_=dr[:])

            # persistent state: ping/pong xh per half, packed cell state
            xh = [
                [
                    const.tile([128, HALF], BF16, tag=f"xh{q}{p}", name=f"xh{q}{p}")
                    for p in range(2)
                ]
                for q in range(2)
            ]
            c2 = const.tile([128, HALF], BF16, tag="c2", name="c2")
            for q in range(2):
                nc.vector.memset(xh[q][0][HID:128, :], 0.0)
            nc.vector.memset(c2[:], 0.0)

            NCH = 2  # free-dim chunks per half (each with its own psum slots)
            CW = HALF // NCH
            NBC = CW // 512
            for t in range(STEP):
                par = t % 2
                x0, x1 = xh[0][par], xh[1][par]
                n0, n1 = xh[0][1 - par], xh[1][1 - par]
                decb = decp.tile([128, HALF], BF16, tag="decb", name="decb")
                nc.sync.dma_start(out=decb[:], in_=dec_d[t])
                nc.sync.dma_start(out=x0[0:DIM, :], in_=x_d[t, :, bass.ts(0, HALF)])
                nc.sync.dma_start(out=x1[0:DIM, :], in_=x_d[t, :, bass.ts(1, HALF)])

                for ch in range(NCH):
                    cs = bass.ds(ch * CW, CW)
                    pif0 = psum.tile([128, CW], F32, tag=f"pA{ch}", name="pif0")
                    for j in range(NBC):
                        js = bass.ds(ch * CW + j * 512, 512)
                        ps = bass.ts(j, 512)
                        nc.tensor.matmul(
                            pif0[:, ps], wfi[:], x0[:, js], start=True, stop=True
                        )
                    tg2 = psum.tile([128, CW], F32, tag=f"pB{ch}", name="tg2")
                    for j in range(NBC):
                        js = bass.ds(ch * CW + j * 512, 512)
                        ps = bass.ts(j, 512)
                        nc.tensor.matmul(
                            tg2[0:HID, ps], wg[:], x1[:, js], start=True, stop=True
                        )
                        nc.tensor.matmul(
                            tg2[HID:128, ps], wg[:], x0[:, js], start=True, stop=True
                        )
                    sif0 = work.tile([128, HALF], BF16, tag="sif0", name="sif0")
                    nc.scalar.activation(
                        sif0[:, cs], pif0[:], AF.Sigmoid, bias=bfi[:]
                    )

                    pif1 = psum.tile([128, CW], F32, tag=f"pA{ch}", name="pif1")
                    for j in range(NBC):
                        js = bass.ds(ch * CW + j * 512, 512)
                        ps = bass.ts(j, 512)
                        nc.tensor.matmul(
                            pif1[:, ps], wif[:], x1[:, js], start=True, stop=True
                        )
                    tgs = work.tile([128, HALF], BF16, tag="tgs", name="tgs")
                    nc.scalar.activation(tgs[:, cs], tg2[:], AF.Tanh, bias=bg[:])

                    poo = psum.tile([128, CW], F32, tag=f"pB{ch}", name="poo")
                    for j in range(NBC):
                        js = bass.ds(ch * CW + j * 512, 512)
                        ps = bass.ts(j, 512)
                        nc.tensor.matmul(
                            poo[0:HID, ps], wo[:], x0[:, js], start=True, stop=True
                        )
                        nc.tensor.matmul(
                            poo[HID:128, ps], wo[:], x1[:, js], start=True, stop=True
                        )
                    sif1 = work.tile([128, HALF], BF16, tag="sif1", name="sif1")
                    nc.scalar.activation(
                        sif1[:, cs], pif1[:], AF.Sigmoid, bias=bif[:]
                    )
                    so2 = work.tile([128, HALF], BF16, tag="so2", name="so2")
                    nc.scalar.activation(so2[:, cs], poo[:], AF.Sigmoid, bias=bo[:])

                    # DVE cell update (bases matched per op)
                    dc2 = work.tile([128, HALF], BF16, tag="dc2", name="dc2")
                    nc.gpsimd.tensor_mul(dc2[:, cs], c2[:, cs], decb[:, cs])
                    igT = work.tile([128, HALF], BF16, tag="igT", name="igT")
                    fdT = work.tile([128, HALF], BF16, tag="fdT", name="fdT")
                    # half0: i at rows 64:128 of sif0, g(h0) at rows 64:128 of tgs
                    nc.vector.tensor_mul(
                        igT[0:HID, cs], sif0[HID:128, cs], tgs[HID:128, cs]
                    )
                    # half1: i at rows 0:64 of sif1, g(h1) at rows 0:64 of tgs
                    nc.vector.tensor_mul(
                        igT[HID:128, cs], sif1[0:HID, cs], tgs[0:HID, cs]
                    )
                    # half0: f at rows 0:64 of sif0, dc at rows 0:64
                    nc.vector.tensor_mul(
                        fdT[0:HID, cs], sif0[0:HID, cs], dc2[0:HID, cs]
                    )
                    # half1: f at rows 64:128 of sif1, dc at rows 64:128
                    nc.vector.tensor_mul(
                        fdT[HID:128, cs], sif1[HID:128, cs], dc2[HID:128, cs]
                    )
                    nc.vector.tensor_add(c2[:, cs], igT[:, cs], fdT[:, cs])
                    tch = work.tile([128, HALF], BF16, tag="tch", name="tch")
                    nc.scalar.activation(tch[:, cs], c2[:, cs], AF.Tanh)
                    nc.vector.tensor_mul(
                        n0[HID:128, cs], so2[0:HID, cs], tch[0:HID, cs]
                    )
                    nc.vector.tensor_mul(
                        n1[HID:128, cs], so2[HID:128, cs], tch[HID:128, cs]
                    )

            # ---- final: q = 1 - sigmoid(h@w + b), noisy-OR over nodules ----
            fpar = STEP % 2
            nb2 = const.tile([1, 1], F32, tag="nb2", name="nb2")
            nc.vector.memset(nb2[:], -fc2_b)
            qall = const.tile([1, BL], F32, tag="qall", name="qall")
            hfin = [
                const.tile([HID, HALF], BF16, tag=f"hf{q}", name=f"hf{q}")
                for q in range(2)
            ]
            for q in range(2):
                nc.vector.tensor_copy(hfin[q][:], xh[q][fpar][HID:128, :])
                for j in range(NB):
                    js = bass.ts(j, 512)
                    pz = psum.tile([1, 512], F32, tag="pA0", name="pz")
                    nc.tensor.matmul(
                        pz[:], fc2[:], hfin[q][:, js], start=True, stop=True
                    )
                    nc.scalar.activation(
                        qall[0:1, bass.ds(q * HALF + j * 512, 512)],
                        pz[:],
                        AF.Sigmoid,
                        scale=-1.0,
                        bias=nb2[:],
                    )
            # product over the 8 nodules (innermost in sample order)
            q3 = qall[0:1].rearrange("p (b n) -> p b n", n=NNOD)
            t1 = const.tile([1, BL // 2], F32, tag="t1", name="t1")
            t13 = t1[0:1].rearrange("p (b n) -> p b n", n=4)
            nc.vector.tensor_mul(t13[:, :, :], q3[:, :, 0:4], q3[:, :, 4:8])
            t2 = const.tile([1, BL // 4], F32, tag="t2", name="t2")
            t23 = t2[0:1].rearrange("p (b n) -> p b n", n=2)
            nc.vector.tensor_mul(t23[:, :, :], t13[:, :, 0:2], t13[:, :, 2:4])
            t3 = const.tile([1, BL // 8], F32, tag="t3", name="t3")
            t33 = t3[0:1].rearrange("p (b n) -> p b n", n=1)
            nc.vector.tensor_mul(t33[:, :, :], t23[:, :, 0:1], t23[:, :, 1:2])
            pred = const.tile([1, BSIZE // NCORES], F32, tag="pred", name="pred")
            nc.vector.tensor_scalar(
                out=pred[:],
                in0=t3[:],
                scalar1=-k_base,
                scalar2=1.0,
                op0=mybir.AluOpType.mult,
                op1=mybir.AluOpType.add,
            )
            nc.sync.dma_start(out=out_d[:], in_=pred[:])

    _split_multiwaits(nc)
    return nc


def kernel(input, time_dis, w_ih, w_hh, b_ih, b_hh, fc2_w, fc2_b, baseline):
    input = np.asarray(input, dtype=np.float32)
    time_dis = np.asarray(time_dis, dtype=np.float32)
    w_ih = np.asarray(w_ih, dtype=np.float32)
    w_hh = np.asarray(w_hh, dtype=np.float32)
    b_ih = np.asarray(b_ih, dtype=np.float32)
    b_hh = np.asarray(b_hh, dtype=np.float32)
    fc2_w = np.asarray(fc2_w, dtype=np.float32)
    fc2_b = np.asarray(fc2_b, dtype=np.float32)
    baseline = np.asarray(baseline, dtype=np.float32)

    bf = ml_dtypes.bfloat16
    bper = BSIZE // NCORES  # 512

    # gates^T = W^T.T @ [x;h], W = [w_ih | w_hh]  [256, 128]
    W = np.concatenate([w_ih, w_hh], axis=1)  # [256, 128]
    lhsT = np.ascontiguousarray(W.T)  # [128, 256] cols: i(0:64) f g o
    li, lf = lhsT[:, 0:64], lhsT[:, 64:128]
    lg, lo = lhsT[:, 128:192], lhsT[:, 192:256]
    wfi = np.ascontiguousarray(np.concatenate([lf, li], axis=1)).astype(bf)
    wif = np.ascontiguousarray(np.concatenate([li, lf], axis=1)).astype(bf)
    wg = np.ascontiguousarray(lg).astype(bf)
    wo = np.ascontiguousarray(lo).astype(bf)
    bias = (b_ih + b_hh).astype(np.float32)
    bi, bfg = bias[0:64], bias[64:128]
    bgg, bog = bias[128:192], bias[192:256]
    bfi = np.ascontiguousarray(np.concatenate([bfg, bi])[:, None])
    bif = np.ascontiguousarray(np.concatenate([bi, bfg])[:, None])
    bg = np.ascontiguousarray(np.concatenate([bgg, bgg])[:, None])
    bo = np.ascontiguousarray(np.concatenate([bog, bog])[:, None])
    fc2w = np.ascontiguousarray(fc2_w.reshape(1, HID).T).astype(bf)  # [64,1]
    k_base = float(1.0 - 1.0 / (1.0 + math.exp(-float(baseline[0]))))

    nc = _build(float(fc2_b[0]), k_base)

    in_maps = []
    for k in range(NCORES):
        bs = slice(k * bper, (k + 1) * bper)
        xs = input[:, bs].reshape(STEP, BL, DIM)
        xs = np.ascontiguousarray(xs.transpose(0, 2, 1)).astype(bf)  # [S,64,BL]
        td = time_dis[bs]  # [512, 32]
        td_bn = np.repeat(td.T, NNOD, axis=1)  # [32, 4096] sample-major
        td_used = np.concatenate([td_bn[:1], td_bn[:-1]], axis=0)
        dec = (1.0 / np.log(math.e + td_used)).astype(bf)  # [32, BL]
        # dec2[t, 0:64, j] = dec[t, j] (half0) ; dec2[t, 64:128, j] = dec[t, HALF+j]
        dec2 = np.empty((STEP, 128, HALF), dtype=bf)
        dec2[:, 0:HID, :] = dec[:, None, 0:HALF]
        dec2[:, HID:128, :] = dec[:, None, HALF:BL]
        in_maps.append(
            {
                "x": xs,
                "dec": dec2,
                "wfi": wfi,
                "wif": wif,
                "wg": wg,
                "wo": wo,
                "bfi": bfi,
                "bif": bif,
                "bg": bg,
                "bo": bo,
                "fc2w": fc2w,
            }
        )

    res = None
    last_err = None
    for _attempt in range(3):
        try:
            res = run_bass_kernel_spmd(nc, in_maps, list(range(NCORES)))
            break
        except Exception as e:  # transient NRT device errors recover on retry
            last_err = e
    if res is None:
        raise last_err
    global LAST_RESULT
    LAST_RESULT = res
    out = np.concatenate(
        [np.asarray(res.results[k]["out"]).reshape(bper) for k in range(NCORES)]
    )
    return out.astype(np.float32)

